# revision 9
# baseline (speedup 1.0000x reference)
"""Multi-head attention (B=2, S=2048, D=1024, H=16, K=64) on 8 TRN2 cores.

Sharding: core c -> batch b=c//4, head-group g=c%4 (4 heads, 256-wide slice
of Wq/Wk/Wv columns and Wo rows).  Each core computes a partial (2048, 1024)
output; host sums groups of 4 cores and adds bo.

Per-core layout (all transposed so no on-chip transposes are needed):
  - host supplies xT = x[b].T  (D, S)
  - Q^T, K^T computed as [gw_col, S] via lhsT=W chunk, rhs=xT chunk
  - scores^T[j, i] via lhsT=K^T chunk, rhs=Q^T  -> softmax denom via a ones
    column appended to V (V_aug), probs^T = exp(scores^T / 8) with no max
    subtraction (scores are ~N(0,1); exp cannot overflow fp32)
  - O^T = V_aug^T @ probs^T, divided by denom, feeds Wo matmul as stationary
"""

import os
import sys
from contextlib import ExitStack

import numpy as np

if "/opt/trn_rl_repo" not in sys.path:
    sys.path.insert(0, "/opt/trn_rl_repo")

import concourse.bass as bass
import concourse.mybir as mybir
import concourse.tile as tile
from concourse import bacc
from concourse.bass import ds, ts
from concourse.bass_utils import run_bass_kernel_spmd

B, S, D = 2, 2048, 1024
H, KS = 16, 64
NCORES = 8
HPC = H // 4          # 4 heads per core
GW = HPC * KS         # 256-wide head-group slice
P = 128
ND = D // P           # 8 contraction chunks over d_model
NM = GW // P          # 2 col chunks of the group slice
NI = 4                # i-groups
IT = S // NI          # 512 rows per i-group
NJ = S // P           # 16 j-chunks
NO = D // 512         # 2 out-col groups for Wo

F32 = mybir.dt.float32
F32R = mybir.dt.float32r
USE_F32R = True
MMDT = F32R if USE_F32R else F32
EXP = mybir.ActivationFunctionType.Exp


def _mm(ap):
    return ap


def _mha_core(tc, out, xT, wq, wk, wv, wo, bq, bk, bv):
    nc = tc.nc
    with ExitStack() as ctx:
        cp = ctx.enter_context(tc.tile_pool(name="const", bufs=1))
        probs_pool = ctx.enter_context(tc.tile_pool(name="probs", bufs=3))
        out_pool = ctx.enter_context(tc.tile_pool(name="outsb", bufs=3))
        ps_acc = ctx.enter_context(tc.tile_pool(name="ps_acc", bufs=2, space="PSUM"))
        ps_s = ctx.enter_context(tc.tile_pool(name="ps_s", bufs=2, space="PSUM"))
        ps_o = ctx.enter_context(tc.tile_pool(name="ps_o", bufs=2, space="PSUM"))
        ps_w = ctx.enter_context(tc.tile_pool(name="ps_w", bufs=1, space="PSUM"))
        ps_c = ctx.enter_context(tc.tile_pool(name="ps_c", bufs=1, space="PSUM"))

        # ---- constants / inputs to SBUF ----
        xT_sb = cp.tile([P, ND, S], MMDT)
        for dc in range(ND):
            eng = nc.sync if dc % 2 == 0 else nc.gpsimd
            eng.dma_start(xT_sb[:, dc, :], xT[ts(dc, P), :])
        wq_sb = cp.tile([P, ND, GW], MMDT)
        wk_sb = cp.tile([P, ND, GW], MMDT)
        wv_sb = cp.tile([P, ND, GW], MMDT)
        nc.gpsimd.dma_start(wq_sb[:], wq.rearrange("(nd p) n -> p nd n", p=P))
        nc.gpsimd.dma_start(wk_sb[:], wk.rearrange("(nd p) n -> p nd n", p=P))
        nc.gpsimd.dma_start(wv_sb[:], wv.rearrange("(nd p) n -> p nd n", p=P))
        wo_sb = cp.tile([P, NM, D], MMDT)
        nc.gpsimd.dma_start(wo_sb[:], wo.rearrange("(nm p) n -> p nm n", p=P))
        bq_sb = cp.tile([P, NM], F32)
        bk_sb = cp.tile([P, NM], F32)
        nc.sync.dma_start(bq_sb[:], bq.rearrange("(m p) -> p m", p=P))
        nc.sync.dma_start(bk_sb[:], bk.rearrange("(m p) -> p m", p=P))
        bv_bc = cp.tile([P, GW], F32)
        nc.sync.dma_start(bv_bc[:], bv.partition_broadcast(P))
        ones_f32 = cp.tile([P, HPC * NJ], F32)
        nc.vector.memset(ones_f32[:], 1.0)
        ones64 = cp.tile([1, KS], MMDT)
        nc.scalar.copy(ones64[:], ones_f32[ds(0, 1), 0:KS])

        QT = cp.tile([P, NM, S], MMDT)
        KT = cp.tile([P, NM, S], MMDT)
        OT = cp.tile([P, NM, S], MMDT)
        # V_aug[:, h, jt, 0:64] = V rows, [:, h, jt, 64] = 1.0 (denominator col)
        V_aug = cp.tile([P, HPC, NJ, KS + 1], MMDT)
        nc.scalar.copy(
            V_aug[:, :, :, ds(KS, 1)].rearrange("p h j o -> p (h j o)"),
            ones_f32[:])

        # ---- Q^T / K^T projections: [gw_col, S] ----
        for m in range(NM):
            for ig in range(NI):
                qt_ps = ps_acc.tile([P, IT], F32, tag="acc")
                kt_ps = ps_acc.tile([P, IT], F32, tag="acc")
                for dc in range(ND):
                    nc.tensor.matmul(
                        qt_ps[:],
                        _mm(wq_sb[:, dc, ts(m, P)]),
                        _mm(xT_sb[:, dc, ts(ig, IT)]),
                        start=(dc == 0), stop=(dc == ND - 1),
                    )
                for dc in range(ND):
                    nc.tensor.matmul(
                        kt_ps[:],
                        _mm(wk_sb[:, dc, ts(m, P)]),
                        _mm(xT_sb[:, dc, ts(ig, IT)]),
                        start=(dc == 0), stop=(dc == ND - 1),
                    )
                nc.vector.tensor_scalar_add(
                    QT[:, m, ts(ig, IT)], qt_ps[:], bq_sb[:, ds(m, 1)])
                nc.vector.tensor_scalar_add(
                    KT[:, m, ts(ig, IT)], kt_ps[:], bk_sb[:, ds(m, 1)])

        # ---- V projection (natural layout) + bias + ones col ----
        for jt in range(NJ):
            v_ps = ps_acc.tile([P, IT], F32, tag="acc")
            for dc in range(ND):
                nc.tensor.matmul(
                    v_ps[:, 0:GW],
                    _mm(xT_sb[:, dc, ts(jt, P)]),
                    _mm(wv_sb[:, dc, :]),
                    start=(dc == 0), stop=(dc == ND - 1),
                )
            nc.vector.tensor_add(
                V_aug[:, :, jt, 0:KS],
                v_ps[:, 0:GW].rearrange("p (h k) -> p h k", h=HPC),
                bv_bc[:].rearrange("p (h k) -> p h k", h=HPC),
            )

        # ---- attention + output projection, i-group major ----
        for ig in range(NI):
            for h in range(HPC):
                po = 64 * (h % 2)   # partition offset of head h within its NM tile
                m = h // 2
                o_ps = ps_o.tile([KS + 1, IT], F32)
                for jc in range(NJ):
                    s_ps = ps_s.tile([P, IT], F32)
                    nc.tensor.matmul(
                        s_ps[:],
                        _mm(KT[ds(po, KS), m, ts(jc, P)]),
                        _mm(QT[ds(po, KS), m, ts(ig, IT)]),
                        start=True, stop=True,
                    )
                    pt = probs_pool.tile([P, IT], MMDT)
                    nc.scalar.activation(pt[:], s_ps[:], EXP, scale=0.125)
                    nc.tensor.matmul(
                        o_ps[:],
                        _mm(V_aug[:, h, jc, :]),
                        _mm(pt[:]),
                        start=(jc == 0), stop=(jc == NJ - 1),
                    )
                recip = out_pool.tile([1, IT], MMDT)
                with nc.allow_low_precision(reason="f32r has f32 bits"):
                    nc.vector.reciprocal(recip[:], o_ps[ds(KS, 1), :])
                bc_ps = ps_c.tile([KS, IT], F32)
                nc.tensor.matmul(bc_ps[:], _mm(ones64[:]), _mm(recip[:]),
                                 start=True, stop=True)
                bc_sb = out_pool.tile([KS, IT], F32)
                nc.scalar.copy(bc_sb[:], bc_ps[:])
                nc.vector.tensor_mul(
                    OT[ds(po, KS), m, ts(ig, IT)], o_ps[ds(0, KS), :], bc_sb[:])

            # Wo partial for the 4 row-tiles of this i-group
            for itl in range(NI):
                it = ig * NI + itl
                for ncol in range(NO):
                    w_ps = ps_w.tile([P, 512], F32)
                    for hc in range(NM):
                        nc.tensor.matmul(
                            w_ps[:],
                            _mm(OT[:, hc, ts(it, P)]),
                            _mm(wo_sb[:, hc, ts(ncol, 512)]),
                            start=(hc == 0), stop=(hc == NM - 1),
                        )
                    o_sb = out_pool.tile([P, 512], F32)
                    nc.vector.tensor_copy(o_sb[:], w_ps[:])
                    eng = nc.sync if (it + ncol) % 2 == 0 else nc.gpsimd
                    eng.dma_start(out[ts(it, P), ts(ncol, 512)], o_sb[:])


def _build_program():
    nc = bacc.Bacc("TRN2", target_bir_lowering=False, debug=False,
                   num_devices=NCORES)
    xT = nc.dram_tensor("xT", (D, S), MMDT, kind="ExternalInput").ap()
    wq = nc.dram_tensor("wq", (D, GW), MMDT, kind="ExternalInput").ap()
    wk = nc.dram_tensor("wk", (D, GW), MMDT, kind="ExternalInput").ap()
    wv = nc.dram_tensor("wv", (D, GW), MMDT, kind="ExternalInput").ap()
    wo = nc.dram_tensor("wo", (GW, D), MMDT, kind="ExternalInput").ap()
    bq = nc.dram_tensor("bq", (GW,), F32, kind="ExternalInput").ap()
    bk = nc.dram_tensor("bk", (GW,), F32, kind="ExternalInput").ap()
    bv = nc.dram_tensor("bv", (GW,), F32, kind="ExternalInput").ap()
    out = nc.dram_tensor("out", (S, D), F32, kind="ExternalOutput").ap()
    with tile.TileContext(nc) as tc:
        _mha_core(tc, out, xT, wq, wk, wv, wo, bq, bk, bv)
    nc.compile()
    return nc


_program = None


def _get_program():
    global _program
    if _program is None:
        _program = _build_program()
    return _program


def make_in_maps(x, Wq, bq, Wk, bk, Wv, bv, Wo, bo):
    in_maps = []
    f = np.float32
    for c in range(NCORES):
        b, g = divmod(c, 4)
        sl = slice(g * GW, (g + 1) * GW)
        in_maps.append({
            "xT": np.ascontiguousarray(x[b].T, dtype=f),
            "wq": np.ascontiguousarray(Wq[:, sl], dtype=f),
            "wk": np.ascontiguousarray(Wk[:, sl], dtype=f),
            "wv": np.ascontiguousarray(Wv[:, sl], dtype=f),
            "wo": np.ascontiguousarray(Wo[sl, :], dtype=f),
            "bq": np.ascontiguousarray(bq[sl], dtype=f),
            "bk": np.ascontiguousarray(bk[sl], dtype=f),
            "bv": np.ascontiguousarray(bv[sl], dtype=f),
        })
    return in_maps


def run(inputs, trace=False):
    nc = _get_program()
    in_maps = make_in_maps(**inputs)
    res = run_bass_kernel_spmd(nc, in_maps, core_ids=list(range(NCORES)),
                               trace=trace)
    bo = inputs["bo"].astype(np.float32)
    parts = [res.results[c]["out"] for c in range(NCORES)]
    y = np.stack(
        [parts[4 * b] + parts[4 * b + 1] + parts[4 * b + 2] + parts[4 * b + 3] + bo
         for b in range(B)], axis=0)
    return y.astype(np.float32), res


def kernel(**inputs):
    y, _ = run(inputs, trace=False)
    return y
